# revision 31
# baseline (speedup 1.0000x reference)
"""Trainium2 Bass kernel for a post-LN transformer block.

Reference computation (per batch element):
  q,k,v = per-head projections of x            [T,D] x [H,D,HS]
  attn  = softmax(causal(q k^T / sqrt(HS)))
  o     = attn @ v, concat heads, @ Wp
  x     = LN(o + x)
  h     = gelu(x @ W1) @ W2
  out   = LN(h + x)

Sharding: pure data-parallel over batch. B=16 across 8 cores -> 2 batch
elements per core, weights replicated, no collectives.

Engine-balance strategy (per core):
  - softmax exp is the dominant cost (~T^2/2 * H elements). It is split
    between ACT (native Exp) and DVE (Schraudolph bit-trick exp: bf16 bits
    = int16(round(s * 2^7*log2(e)/4 + 127*128 - C)); f32->int16 convert
    saturates, so extreme scores land at -0.0).  Final-output rel err
    contribution measured ~5e-4 (residual path dilutes attention error).
  - all transposes (xT, oT, x1T) run as XBAR block-transposes on the DMA
    engines (SBUF->SBUF, bf16), not on PE+DVE.
  - x -> bf16 conversion is a casting SWDGE DMA on gpsimd.
  - LN applies and the final output scaling run on Pool (gpsimd).
  - causal diag-block mask: gpsimd affine_select (fill=0) on E after exp.
  - scores S^T per (head-group, u-chunk) with 4-way PE row tiling
    (tile_position=(32j,0), K=16); av uses the ones-column trick so
    softmax denominators fall out of the attention matmul.
"""

import sys
from contextlib import ExitStack

import numpy as np

for _p in ("/opt/trn_rl_repo", "/opt/pypackages"):
    if _p not in sys.path:
        sys.path.append(_p)

import ml_dtypes  # noqa: E402

import concourse.bacc as bacc  # noqa: E402
import concourse.tile as tile  # noqa: E402
from concourse import mybir  # noqa: E402
from concourse.bass_utils import run_bass_kernel_spmd  # noqa: E402

F32 = mybir.dt.float32
BF16 = mybir.dt.bfloat16
I16 = mybir.dt.int16
I32 = mybir.dt.int32
AF = mybir.ActivationFunctionType
ALU = mybir.AluOpType

B_FULL = 16
N_CORES = 8
B_PER = B_FULL // N_CORES  # 2
T = 1024
D = 128
H = 8
HS = 16
TC = T // 128  # 8 chunks of 128 tokens
G = 2  # head groups of 4 (32-partition strips)
EPS = 1e-5

# Schraudolph exp constants for bf16 bit patterns, including the 1/sqrt(HS)
# score scale: e = bits16(round(s * (2^7/ln2) * 0.25 + (127*128 - C)))
A_SCH = (128.0 / float(np.log(2.0))) * 0.25
B_SCH = 127.0 * 128.0 - 6.0


def _pieces(tcols):
    """Split tcols into chunks of <=512."""
    out = []
    start = 0
    while start < tcols:
        ln = min(512, tcols - start)
        out.append((start, ln))
        start += ln
    return out


def _use_dve_exp(b, g, uc, p):
    """Which exp instructions run on DVE (bit-trick) vs ACT (native Exp).
    The p-halves of every score tile split between the engines so ACT and
    DVE run concurrently within each u-chunk (chunk-level assignment makes
    the engines take turns instead).  Batch 1's uc>=4 run fully on DVE so
    the woven gelu block never interleaves with ACT exp (each interleave
    costs a 1283ns ACT table load).  (g0, uc0)'s p1 stays on ACT to even
    out the engine loads."""
    if b == 1 and uc >= 4:
        return True
    if p == 0:
        return False
    return not (uc == 0 and g == 0)


def build_block_kernel(loop_n=1):
    nc = bacc.Bacc(
        "TRN2",
        target_bir_lowering=False,
        debug=False,
        enable_asserts=False,
    )

    x_dram = nc.dram_tensor("x", [B_PER, T, D], F32, kind="ExternalInput").ap()
    wq_dram = nc.dram_tensor("wq", [D, G * 128], BF16, kind="ExternalInput").ap()
    wk_dram = nc.dram_tensor("wk", [D, G * 128], BF16, kind="ExternalInput").ap()
    wv_dram = nc.dram_tensor("wv", [D, 128], BF16, kind="ExternalInput").ap()
    wp_dram = nc.dram_tensor("wp", [128, D], BF16, kind="ExternalInput").ap()
    w1_dram = nc.dram_tensor("w1", [D, 512], BF16, kind="ExternalInput").ap()
    w2_dram = nc.dram_tensor("w2", [128, 4, D], BF16, kind="ExternalInput").ap()
    out_dram = nc.dram_tensor("out", [B_PER, T, D], F32, kind="ExternalOutput").ap()

    with tile.TileContext(nc) as tc:
        if loop_n == 1:
            with ExitStack() as ctx:
                _body(ctx, tc, x_dram, wq_dram, wk_dram, wv_dram, wp_dram,
                      w1_dram, w2_dram, out_dram)
        else:
            with tc.For_i(0, loop_n, 1):
                with ExitStack() as ctx:
                    _body(ctx, tc, x_dram, wq_dram, wk_dram, wv_dram,
                          wp_dram, w1_dram, w2_dram, out_dram)

    nc.compile()
    return nc


def _body(ctx, tc, x_dram, wq_dram, wk_dram, wv_dram, wp_dram, w1_dram,
          w2_dram, out_dram):
    nc = tc.nc

    const = ctx.enter_context(tc.tile_pool(name="const", bufs=1))
    sb = ctx.enter_context(tc.tile_pool(name="sb", bufs=1))
    eb = ctx.enter_context(tc.tile_pool(name="eb", bufs=1))
    # PSUM budget (8 banks): "s" = 2-bank slots x2 (score tiles; also the
    # [128,1024] q/k/v tiles during attention prep), "hp" = 1 bank x2 (h1
    # halves + proj/x2 outputs), "o" = 1 bank x2 (av accumulators).
    # The MLP/proj tiles deliberately do NOT share "s" so the woven tail of
    # one batch never blocks the other batch's score pipeline.
    ps = ctx.enter_context(tc.tile_pool(name="ps", bufs=1, space="PSUM"))

    # ---- constants ----
    wq_sb = const.tile([D, G * 128], BF16, tag="wq")
    nc.sync.dma_start(wq_sb, wq_dram)
    wk_sb = const.tile([D, G * 128], BF16, tag="wk")
    nc.sync.dma_start(wk_sb, wk_dram)
    wv_sb = const.tile([D, 128], BF16, tag="wv")
    nc.sync.dma_start(wv_sb, wv_dram)
    wp_sb = const.tile([128, D], BF16, tag="wp")
    nc.sync.dma_start(wp_sb, wp_dram)
    w1_sb = const.tile([D, 512], BF16, tag="w1")
    nc.sync.dma_start(w1_sb, w1_dram)
    w2_sb = const.tile([128, 4, D], BF16, tag="w2")
    nc.sync.dma_start(w2_sb, w2_dram)

    # Schraudolph bias tensor for the DVE exp: cols 0:128 carry an extra
    # -1e6 on sub-diagonal positions (t_local < u_local) so the f32->int16
    # saturation maps masked scores to -32768 = bf16 -0.0; cols 128:640 are
    # the plain bias for off-diagonal score pieces.
    maskb = const.tile([128, 2, 640], F32, tag="maskb")
    nc.vector.memset(maskb, B_SCH)
    nc.gpsimd.affine_select(
        out=maskb[:, :, 0:128], in_=maskb[:, :, 0:128],
        pattern=[[0, 2], [1, 128]],
        compare_op=ALU.is_ge, fill=B_SCH - 1.0e6, base=0,
        channel_multiplier=-1,
    )

    # ---- per-batch persistent sbuf ----
    x_td = [sb.tile([128, TC, 128], F32, tag=f"x_td{b}", name=f"x_td{b}")
            for b in range(B_PER)]
    x1bf = [sb.tile([128, TC, 128], BF16, tag=f"x1{b}", name=f"x1{b}")
            for b in range(B_PER)]
    x1T = [sb.tile([128, TC, 128], BF16, tag=f"x1T{b}", name=f"x1T{b}")
           for b in range(B_PER)]
    gT = [sb.tile([128, 4, T], BF16, tag=f"gT{b}", name=f"gT{b}")
          for b in range(B_PER)]
    oT = [sb.tile([128, TC, 128], BF16, tag=f"oT{b}", name=f"oT{b}")
          for b in range(B_PER)]

    xT_all = []

    def load_x(b):
        """x load + bf16 cast + transpose, all on DMA engines; emitted for
        both batches up front so batch 1's chain hides under batch 0."""
        xb = x_dram[b].rearrange("(c p) d -> p c d", p=128)
        nc.sync.dma_start(x_td[b], xb)
        xbf = sb.tile([128, TC, 128], BF16, tag=f"xbf{b}", name=f"xbf{b}")
        nc.gpsimd.dma_start(xbf, xb)
        xT = sb.tile([128, TC, 128], BF16, tag=f"xT{b}", name=f"xT{b}")
        nc.sync.dma_start(xT, xbf, transpose=True)
        xT_all.append(xT)

    def attn_core(b, weave=None):
        """QKV + per-uc (scores, exp, mask, av).  Calls weave(uc) after each
        u-chunk so the other batch's tail work can be interleaved."""
        xTf = xT_all[b].rearrange("p c t -> p (c t)")

        # qT / kT in 32-strip padded layout: head h=4g+j at partitions 32j
        qT = []
        kT = []
        for g in range(G):
            qp = ps.tile([128, T], F32, tag="s", bufs=2)
            for tb in range(2):
                nc.tensor.matmul(
                    qp[:, tb * 512:(tb + 1) * 512],
                    lhsT=wq_sb[:, g * 128:(g + 1) * 128],
                    rhs=xTf[:, tb * 512:(tb + 1) * 512],
                    start=True, stop=True,
                )
            qs = sb.tile([128, T], BF16, tag=f"qT{g}", name=f"qT{g}")
            nc.scalar.copy(out=qs, in_=qp)
            qT.append(qs)
            kp = ps.tile([128, T], F32, tag="s", bufs=2)
            for tb in range(2):
                nc.tensor.matmul(
                    kp[:, tb * 512:(tb + 1) * 512],
                    lhsT=wk_sb[:, g * 128:(g + 1) * 128],
                    rhs=xTf[:, tb * 512:(tb + 1) * 512],
                    start=True, stop=True,
                )
            ks = sb.tile([128, T], BF16, tag=f"kT{g}", name=f"kT{g}")
            # k eviction on ACT (Copy needs no table) to offload DVE
            nc.scalar.copy(out=ks, in_=kp)
            kT.append(ks)

        # v in [t, h*16+s] layout -> v' [u-chunk][h][17] bf16 with ones col
        vq = sb.tile([128, TC, H, 17], BF16, tag="vq")
        nc.vector.memset(vq[:, :, :, 16:17], 1.0)
        vp = ps.tile([128, T], F32, tag="s", bufs=2)
        for c in range(TC):
            # one accumulation group per PSUM bank (4 chunks of 128 cols);
            # start=True zeroes the whole bank, later chunks add onto zeros
            nc.tensor.matmul(
                vp[:, c * 128:(c + 1) * 128],
                lhsT=xTf[:, c * 128:(c + 1) * 128],
                rhs=wv_sb,
                start=(c % 4 == 0), stop=(c % 4 == 3),
                skip_group_check=True,
            )
        vsrc = vp.rearrange("p (c h s) -> p c h s", c=TC, h=H)
        nc.scalar.copy(out=vq[:, :, :, 0:16], in_=vsrc)

        # o (normalized attention output) accumulates here, then XBAR
        # transposes to oT in two 4-chunk pieces
        o_all = sb.tile([128, TC, 128], BF16, tag="o_all")

        E = [[None] * TC for _ in range(G)]

        def s_exp(g, uc):
            t0 = uc * 128
            tcols = T - t0
            e = eb.tile([128, 4, tcols], BF16, tag=f"E{g}_{uc}",
                        name=f"E{g}_{uc}")
            E[g][uc] = e
            for (pofs, plen) in _pieces(tcols):
                # head-pair score tiles: 2 banks each so the "s" tag can
                # double-buffer (scores of the next chunk overlap this exp)
                for p in range(2):
                    dve = _use_dve_exp(b, g, uc, p)
                    sp = ps.tile([128, 2, 512], F32, tag="s", bufs=2)
                    for jj in range(2):
                        j = 2 * p + jj
                        nc.tensor.matmul(
                            sp[:, jj, 0:plen],
                            lhsT=kT[g][32 * j:32 * j + 16, t0:t0 + 128],
                            rhs=qT[g][32 * j:32 * j + 16,
                                      t0 + pofs:t0 + pofs + plen],
                            start=True, stop=True,
                            tile_position=(32 * j, 0),
                        )
                    dst = e[:, 2 * p:2 * p + 2, pofs:pofs + plen]
                    if dve:
                        # bit-trick exp; the bias tensor also applies the
                        # causal diag mask via int16 saturation -> bf16 -0.0
                        bias = (maskb[:, :, 0:plen] if pofs == 0
                                else maskb[:, :, 128:128 + plen])
                        nc.vector.scalar_tensor_tensor(
                            out=dst.bitcast(I16), in0=sp[:, :, 0:plen],
                            scalar=A_SCH, in1=bias,
                            op0=ALU.mult, op1=ALU.add,
                        )
                    else:
                        nc.scalar.activation(
                            out=dst, in_=sp[:, :, 0:plen],
                            func=AF.Exp, scale=0.25,
                        )
                        if pofs == 0:
                            # causal mask on this head-pair's diagonal
                            # 128-block: keep where t_local >= u_local
                            # (partition index); only ACT halves need this
                            # (DVE halves mask via the bias tensor)
                            nc.gpsimd.affine_select(
                                out=e[:, 2 * p:2 * p + 2, 0:128],
                                in_=e[:, 2 * p:2 * p + 2, 0:128],
                                pattern=[[0, 2], [1, 128]],
                                compare_op=ALU.is_ge, fill=0.0, base=0,
                                channel_multiplier=-1,
                            )

        ops = {}

        def av_old(tcb):
            """Attention@v contributions from u-chunks < tcb: these depend
            only on already-finished E chunks, so PE runs them while ACT/DVE
            compute exp(tcb)."""
            op = ps.tile([128, H, 17], F32, tag="o", bufs=2)
            ops[tcb] = op
            for uc in range(tcb):
                ofs = (tcb - uc) * 128
                for g in range(G):
                    for j in range(4):
                        h = 4 * g + j
                        nc.tensor.matmul(
                            op[:, h, :],
                            lhsT=E[g][uc][:, j, ofs:ofs + 128],
                            rhs=vq[:, uc, h, :],
                            start=(uc == 0 and h == 0),
                            stop=False,
                            skip_group_check=True,
                        )

        def av_fin(tcb):
            """Diagonal-chunk contributions (need exp(tcb)+mask) and the
            softmax normalization.  Emitted AFTER the next chunk's scores so
            the in-order PE stream never stalls on exp latency."""
            op = ops.pop(tcb)
            for g in range(G):
                for j in range(4):
                    h = 4 * g + j
                    nc.tensor.matmul(
                        op[:, h, :],
                        lhsT=E[g][tcb][:, j, 0:128],
                        rhs=vq[:, tcb, h, :],
                        start=(tcb == 0 and h == 0),
                        stop=(h == H - 1),
                        skip_group_check=True,
                    )
            recip8 = sb.tile([128, H], F32, tag="recip8")
            nc.vector.reciprocal(recip8, op[:, :, 16])
            o_blk = o_all[:, tcb, :].rearrange("p (h s) -> p h s", h=H)
            nc.vector.tensor_mul(
                o_blk, op[:, :, 0:16], recip8.broadcast_to([128, H, 16])
            )
            if tcb == 3 or tcb == 7:
                c0 = tcb - 3
                nc.sync.dma_start(oT[b][:, c0:c0 + 4, :],
                                  o_all[:, c0:c0 + 4, :], transpose=True)

        for uc in range(TC):
            for g in range(G):
                s_exp(g, uc)
            av_old(uc)
            if uc > 0:
                av_fin(uc - 1)
            if weave is not None:
                weave(uc - 1)
        av_fin(TC - 1)
        if weave is not None:
            weave(TC - 1)

    def rsqrt_rows(vsrc, tagp):
        """rstd = 1/sqrt(vsrc + EPS) on Pool via the f32 bit trick plus two
        Newton steps (final rel err ~4e-6).  Keeps Ln/Exp off ACT: with this
        compiler's activation tables Ln and Exp live in different table sets,
        so each ACT-based rstd cost two 1283ns table loads."""
        ve = sb.tile([128, TC], F32, tag=tagp + "ve", name=tagp + "ve")
        nc.gpsimd.tensor_scalar(out=ve, in0=vsrc, scalar1=1.0, scalar2=EPS,
                                op0=ALU.mult, op1=ALU.add)
        y = sb.tile([128, TC], F32, tag=tagp + "y", name=tagp + "y")
        nc.gpsimd.tensor_scalar(
            out=y.bitcast(I32), in0=ve.bitcast(I32),
            scalar1=-0.5, scalar2=float(0x5F3759DF),
            op0=ALU.mult, op1=ALU.add,
        )
        t = sb.tile([128, TC], F32, tag=tagp + "t", name=tagp + "t")
        for _ in range(2):
            # y <- y * (1.5 - 0.5 * ve * y^2), Pool-supported ops only
            # (scalar_tensor_tensor is DVE-only on this core version)
            nc.gpsimd.tensor_mul(t, y, y)
            nc.gpsimd.tensor_mul(t, t, ve)
            nc.gpsimd.tensor_scalar(
                out=t, in0=t, scalar1=-0.5, scalar2=1.5,
                op0=ALU.mult, op1=ALU.add)
            nc.gpsimd.tensor_mul(y, t, y)
        return y

    def tail_pieces(b):
        """Emission closures for proj+LN1+MLP+LN2, in dependency order."""
        res1 = sb.tile([128, TC, 128], F32, tag="res1", name=f"res1_{b}")
        bn6 = sb.tile([128, TC, 6], F32, tag="bn6", name=f"bn6_{b}")
        mv = sb.tile([128, TC, 2], F32, tag="mv", name=f"mv_{b}")
        res2 = sb.tile([128, TC, 128], F32, tag="res2", name=f"res2_{b}")
        bn6b = sb.tile([128, TC, 6], F32, tag="bn6b", name=f"bn6b_{b}")
        mvb = sb.tile([128, TC, 2], F32, tag="mvb", name=f"mvb_{b}")

        def proj(c0):
            def f():
                for c in range(c0, c0 + 4):
                    pp = ps.tile([128, 128], F32, tag="hp", bufs=2)
                    nc.tensor.matmul(
                        pp, lhsT=oT[b][:, c, :], rhs=wp_sb,
                        start=True, stop=True,
                    )
                    nc.vector.tensor_add(res1[:, c, :], pp, x_td[b][:, c, :])
                    nc.vector.bn_stats(out=bn6[:, c, :], in_=res1[:, c, :])
                    nc.vector.bn_aggr(out=mv[:, c, :], in_=bn6[:, c, :])
            return f

        def ln1_apply():
            rstd8 = rsqrt_rows(mv[:, :, 1], "r1")
            for c in range(TC):
                nc.gpsimd.tensor_scalar(
                    out=x1bf[b][:, c, :], in0=res1[:, c, :],
                    scalar1=mv[:, c, 0:1], scalar2=rstd8[:, c:c + 1],
                    op0=ALU.subtract, op1=ALU.mult,
                )

        def x1t(c0):
            def f():
                nc.sync.dma_start(x1T[b][:, c0:c0 + 4, :],
                                  x1bf[b][:, c0:c0 + 4, :], transpose=True)
            return f

        x1Tf = x1T[b].rearrange("p c t -> p (c t)")

        def h1(fc):
            def f():
                for tb in range(2):
                    hp = ps.tile([128, 512], F32, tag="hp", bufs=2)
                    nc.tensor.matmul(
                        hp,
                        lhsT=w1_sb[:, fc * 128:(fc + 1) * 128],
                        rhs=x1Tf[:, tb * 512:(tb + 1) * 512],
                        start=True, stop=True,
                    )
                    nc.scalar.activation(
                        out=gT[b][:, fc, tb * 512:(tb + 1) * 512], in_=hp,
                        func=AF.Gelu)
            return f

        def x2(c0):
            def f():
                for c in range(c0, c0 + 2):
                    xp = ps.tile([128, 128], F32, tag="hp", bufs=2)
                    for fc in range(4):
                        nc.tensor.matmul(
                            xp,
                            lhsT=gT[b][:, fc, c * 128:(c + 1) * 128],
                            rhs=w2_sb[:, fc, :],
                            start=(fc == 0), stop=(fc == 3),
                        )
                    nc.vector.tensor_add(res2[:, c, :], xp, x1bf[b][:, c, :])
                    nc.vector.bn_stats(out=bn6b[:, c, :], in_=res2[:, c, :])
                    nc.vector.bn_aggr(out=mvb[:, c, :], in_=bn6b[:, c, :])
            return f

        def ln2_out():
            rstd8 = rsqrt_rows(mvb[:, :, 1], "r2")
            out_sb = sb.tile([128, TC, 128], F32, tag="out_sb",
                             name=f"out_sb_{b}")
            for c in range(TC):
                nc.gpsimd.tensor_scalar(
                    out=out_sb[:, c, :], in0=res2[:, c, :],
                    scalar1=mvb[:, c, 0:1], scalar2=rstd8[:, c:c + 1],
                    op0=ALU.subtract, op1=ALU.mult,
                )
            nc.sync.dma_start(
                out_dram[b].rearrange("(c p) d -> p c d", p=128), out_sb)

        return [
            proj(0), proj(4), ln1_apply, x1t(0), x1t(4),
            h1(0), h1(1), h1(2), h1(3),
            x2(0), x2(2), x2(4), x2(6), ln2_out,
        ]

    # pipelined schedule: b0 attention; then b1 attention with b0's tail
    # pieces woven into its per-uc slots (plus b1's first proj block, whose
    # oT half is ready after b1's av(3)); then b0 leftovers + b1 tail.
    load_x(0)
    load_x(1)
    attn_core(0)
    t0 = tail_pieces(0)
    t1 = tail_pieces(1)
    sched = {0: [0], 1: [1], 2: [2], 3: [3], 4: [4, 5], 5: [6, 7],
             6: [8, 9], 7: [10, 11]}
    done = set()

    def weave(uc):
        for i in sched.get(uc, []):
            t0[i]()
            done.add(i)
        if uc == 5:
            t1[0]()  # b1 proj(0): oT[b1][0:4] ready after b1's av(3)

    attn_core(1, weave=weave)
    for i in range(len(t0)):
        if i not in done:
            t0[i]()
    for i, piece in enumerate(t1):
        if i != 0:
            piece()


# ---------------- host side ----------------

_CACHED = None


def _get_compiled():
    global _CACHED
    if _CACHED is None:
        _CACHED = build_block_kernel()
    return _CACHED


def _prep_weights(inputs):
    f32 = np.float32
    Wq = np.asarray(inputs["Wq"], f32)  # [H, D, HS]
    Wk = np.asarray(inputs["Wk"], f32)
    Wv = np.asarray(inputs["Wv"], f32)
    Wp = np.asarray(inputs["Wp"], f32)  # [H*HS, D]
    W1 = np.asarray(inputs["W1"], f32)  # [D, 4D]
    W2 = np.asarray(inputs["W2"], f32)  # [4D, D]

    bf16 = ml_dtypes.bfloat16

    def strip_pack(W):
        out = np.zeros((D, G * 128), f32)
        for h in range(H):
            g, j = divmod(h, 4)
            out[:, g * 128 + 32 * j: g * 128 + 32 * j + HS] = W[h]
        return out.astype(bf16)

    wq = strip_pack(Wq)
    wk = strip_pack(Wk)
    wv = Wv.transpose(1, 0, 2).reshape(D, H * HS).astype(bf16)
    w2 = W2.reshape(4, 128, D).transpose(1, 0, 2).astype(bf16)
    return {
        "wq": wq, "wk": wk, "wv": np.ascontiguousarray(wv),
        "wp": np.ascontiguousarray(Wp.astype(bf16)),
        "w1": np.ascontiguousarray(W1.astype(bf16)),
        "w2": np.ascontiguousarray(w2),
    }


def run(inputs, trace=False):
    x = np.asarray(inputs["x"], np.float32)
    assert x.shape == (B_FULL, T, D), x.shape
    w = _prep_weights(inputs)
    nc = _get_compiled()
    in_maps = []
    for c in range(N_CORES):
        m = {"x": np.ascontiguousarray(x[c * B_PER:(c + 1) * B_PER])}
        m.update(w)
        in_maps.append(m)
    res = run_bass_kernel_spmd(
        nc, in_maps, core_ids=list(range(N_CORES)), trace=trace
    )
    out = np.concatenate([res.results[c]["out"] for c in range(N_CORES)], axis=0)
    return out.astype(np.float32), res


def kernel(**inputs):
    out, _ = run(inputs)
    return out


def _make_timed_runner(nc, in_maps):
    """Cached single-exec jitted runner with device-resident inputs.
    Returns a zero-arg callable that executes the NEFF once and blocks."""
    import jax
    from jax.experimental.shard_map import shard_map
    from jax.sharding import Mesh, NamedSharding, PartitionSpec

    from concourse import bass2jax, mybir as mb

    bass2jax.install_neuronx_cc_hook()
    partition_name = (
        nc.partition_id_tensor.name if nc.partition_id_tensor else None
    )
    in_names, out_names, out_avals, zero_outs = [], [], [], []
    for alloc in nc.m.functions[0].allocations:
        if not isinstance(alloc, mb.MemoryLocationSet):
            continue
        name = alloc.memorylocations[0].name
        if alloc.kind == "ExternalInput":
            if name != partition_name:
                in_names.append(name)
        elif alloc.kind == "ExternalOutput":
            shape = tuple(alloc.tensor_shape)
            dtype = mb.dt.np(alloc.dtype)
            out_names.append(name)
            out_avals.append(jax.core.ShapedArray(shape, dtype))
            zero_outs.append(np.zeros(shape, dtype))
    n_params = len(in_names)
    bind_names = tuple(in_names + out_names + (
        [partition_name] if partition_name else []))

    def _body(*args):
        operands = list(args)
        if partition_name is not None:
            operands.append(bass2jax.partition_id_tensor())
        return tuple(bass2jax._bass_exec_p.bind(
            *operands,
            out_avals=tuple(out_avals),
            in_names=bind_names,
            out_names=tuple(out_names),
            lowering_input_output_aliases=(),
            sim_require_finite=False,
            sim_require_nnan=False,
            nc=nc,
        ))

    n_cores = len(in_maps)
    devices = jax.devices()[:n_cores]
    mesh = Mesh(np.asarray(devices), ("core",))
    nin = n_params + len(out_names)
    fn = jax.jit(shard_map(
        _body, mesh=mesh,
        in_specs=(PartitionSpec("core"),) * nin,
        out_specs=(PartitionSpec("core"),) * len(out_names),
        check_rep=False,
    ))
    sharding = NamedSharding(mesh, PartitionSpec("core"))
    dev_args = [
        jax.device_put(
            np.concatenate([np.asarray(in_maps[c][nm]) for c in
                            range(n_cores)], axis=0), sharding)
        for nm in in_names
    ] + [
        jax.device_put(
            np.zeros((n_cores * z.shape[0], *z.shape[1:]), z.dtype), sharding)
        for z in zero_outs
    ]

    def call():
        out = fn(*dev_args)
        jax.block_until_ready(out)
        return out

    return call


def bench_ns(inputs, reps=20, loop_a=1, loop_b=33):
    """Per-exec NEFF time measured on device: the kernel body runs inside a
    Tile For_i loop; difference two loop counts to cancel the RPC floor."""
    import time as _time

    x = np.asarray(inputs["x"], np.float32)
    w = _prep_weights(inputs)
    in_maps = []
    for c in range(N_CORES):
        m = {"x": np.ascontiguousarray(x[c * B_PER:(c + 1) * B_PER])}
        m.update(w)
        in_maps.append(m)

    def timeit(call):
        call()
        call()
        best = float("inf")
        vals = []
        for _ in range(reps):
            t0 = _time.perf_counter()
            call()
            dt = _time.perf_counter() - t0
            vals.append(dt)
            best = min(best, dt)
        return best, sorted(vals)

    walls = {}
    for loop_n in (loop_a, loop_b):
        nc = build_block_kernel(loop_n=loop_n)
        call = _make_timed_runner(nc, in_maps)
        walls[loop_n], _ = timeit(call)
    ns = (walls[loop_b] - walls[loop_a]) / (loop_b - loop_a) * 1e9
    return ns, walls


# revision 32
# speedup vs baseline: 6.5094x; 6.5094x over previous
"""Trainium2 Bass kernel for a post-LN transformer block.

Reference computation (per batch element):
  q,k,v = per-head projections of x            [T,D] x [H,D,HS]
  attn  = softmax(causal(q k^T / sqrt(HS)))
  o     = attn @ v, concat heads, @ Wp
  x     = LN(o + x)
  h     = gelu(x @ W1) @ W2
  out   = LN(h + x)

Sharding: pure data-parallel over batch. B=16 across 8 cores -> 2 batch
elements per core, weights replicated, no collectives.

Engine-balance strategy (per core):
  - softmax exp is the dominant cost (~T^2/2 * H elements). It is split
    between ACT (native Exp) and DVE (Schraudolph bit-trick exp: bf16 bits
    = int16(round(s * 2^7*log2(e)/4 + 127*128 - C)); f32->int16 convert
    saturates, so extreme scores land at -0.0).  Final-output rel err
    contribution measured ~5e-4 (residual path dilutes attention error).
  - all transposes (xT, oT, x1T) run as XBAR block-transposes on the DMA
    engines (SBUF->SBUF, bf16), not on PE+DVE.
  - x -> bf16 conversion is a casting SWDGE DMA on gpsimd.
  - LN applies and the final output scaling run on Pool (gpsimd).
  - causal diag-block mask: gpsimd affine_select (fill=0) on E after exp.
  - scores S^T per (head-group, u-chunk) with 4-way PE row tiling
    (tile_position=(32j,0), K=16); av uses the ones-column trick so
    softmax denominators fall out of the attention matmul.
"""

import sys
from contextlib import ExitStack

import numpy as np

for _p in ("/opt/trn_rl_repo", "/opt/pypackages"):
    if _p not in sys.path:
        sys.path.append(_p)

import ml_dtypes  # noqa: E402

import concourse.bacc as bacc  # noqa: E402
import concourse.tile as tile  # noqa: E402
from concourse import mybir  # noqa: E402
from concourse.bass_utils import run_bass_kernel_spmd  # noqa: E402

F32 = mybir.dt.float32
BF16 = mybir.dt.bfloat16
I16 = mybir.dt.int16
I32 = mybir.dt.int32
AF = mybir.ActivationFunctionType
ALU = mybir.AluOpType

B_FULL = 16
N_CORES = 8
B_PER = B_FULL // N_CORES  # 2
T = 1024
D = 128
H = 8
HS = 16
TC = T // 128  # 8 chunks of 128 tokens
G = 2  # head groups of 4 (32-partition strips)
EPS = 1e-5

# Schraudolph exp constants for bf16 bit patterns, including the 1/sqrt(HS)
# score scale: e = bits16(round(s * (2^7/ln2) * 0.25 + (127*128 - C)))
A_SCH = (128.0 / float(np.log(2.0))) * 0.25
B_SCH = 127.0 * 128.0 - 6.0


def _pieces(tcols):
    """Split tcols into chunks of <=512."""
    out = []
    start = 0
    while start < tcols:
        ln = min(512, tcols - start)
        out.append((start, ln))
        start += ln
    return out


def _use_dve_exp(b, g, uc, p):
    """Which exp instructions run on DVE (bit-trick) vs ACT (native Exp).
    The p-halves of every score tile split between the engines so ACT and
    DVE run concurrently within each u-chunk (chunk-level assignment makes
    the engines take turns instead).  Batch 1's uc>=4 run fully on DVE so
    the woven gelu block never interleaves with ACT exp (each interleave
    costs a 1283ns ACT table load).  (g0, uc0)'s p1 stays on ACT to even
    out the engine loads."""
    if b == 1 and uc >= 4:
        return True
    if p == 0:
        return False
    return not (uc == 0 and g == 0)


def build_block_kernel(loop_n=1):
    nc = bacc.Bacc(
        "TRN2",
        target_bir_lowering=False,
        debug=False,
        enable_asserts=False,
    )

    x_dram = nc.dram_tensor("x", [B_PER, T, D], F32, kind="ExternalInput").ap()
    wq_dram = nc.dram_tensor("wq", [D, G * 128], BF16, kind="ExternalInput").ap()
    wk_dram = nc.dram_tensor("wk", [D, G * 128], BF16, kind="ExternalInput").ap()
    wv_dram = nc.dram_tensor("wv", [D, 128], BF16, kind="ExternalInput").ap()
    wp_dram = nc.dram_tensor("wp", [128, D], BF16, kind="ExternalInput").ap()
    w1_dram = nc.dram_tensor("w1", [D, 512], BF16, kind="ExternalInput").ap()
    w2_dram = nc.dram_tensor("w2", [128, 4, D], BF16, kind="ExternalInput").ap()
    out_dram = nc.dram_tensor("out", [B_PER, T, D], F32, kind="ExternalOutput").ap()

    with tile.TileContext(nc) as tc:
        if loop_n == 1:
            with ExitStack() as ctx:
                _body(ctx, tc, x_dram, wq_dram, wk_dram, wv_dram, wp_dram,
                      w1_dram, w2_dram, out_dram)
        else:
            with tc.For_i(0, loop_n, 1):
                with ExitStack() as ctx:
                    _body(ctx, tc, x_dram, wq_dram, wk_dram, wv_dram,
                          wp_dram, w1_dram, w2_dram, out_dram)

    nc.compile()
    return nc


def _body(ctx, tc, x_dram, wq_dram, wk_dram, wv_dram, wp_dram, w1_dram,
          w2_dram, out_dram):
    nc = tc.nc

    const = ctx.enter_context(tc.tile_pool(name="const", bufs=1))
    sb = ctx.enter_context(tc.tile_pool(name="sb", bufs=1))
    eb = ctx.enter_context(tc.tile_pool(name="eb", bufs=1))
    # PSUM budget (8 banks): "s" = 2-bank slots x2 (score tiles; also the
    # [128,1024] q/k/v tiles during attention prep), "hp" = 1 bank x2 (h1
    # halves + proj/x2 outputs), "o" = 1 bank x2 (av accumulators).
    # The MLP/proj tiles deliberately do NOT share "s" so the woven tail of
    # one batch never blocks the other batch's score pipeline.
    ps = ctx.enter_context(tc.tile_pool(name="ps", bufs=1, space="PSUM"))

    # ---- constants ----
    wq_sb = const.tile([D, G * 128], BF16, tag="wq")
    nc.sync.dma_start(wq_sb, wq_dram)
    wk_sb = const.tile([D, G * 128], BF16, tag="wk")
    nc.sync.dma_start(wk_sb, wk_dram)
    wv_sb = const.tile([D, 128], BF16, tag="wv")
    nc.sync.dma_start(wv_sb, wv_dram)
    wp_sb = const.tile([128, D], BF16, tag="wp")
    nc.sync.dma_start(wp_sb, wp_dram)
    w1_sb = const.tile([D, 512], BF16, tag="w1")
    nc.sync.dma_start(w1_sb, w1_dram)
    w2_sb = const.tile([128, 4, D], BF16, tag="w2")
    nc.sync.dma_start(w2_sb, w2_dram)

    # Schraudolph bias tensor for the DVE exp: cols 0:128 carry an extra
    # -1e6 on sub-diagonal positions (t_local < u_local) so the f32->int16
    # saturation maps masked scores to -32768 = bf16 -0.0; cols 128:640 are
    # the plain bias for off-diagonal score pieces.
    maskb = const.tile([128, 2, 640], F32, tag="maskb")
    nc.vector.memset(maskb, B_SCH)
    nc.gpsimd.affine_select(
        out=maskb[:, :, 0:128], in_=maskb[:, :, 0:128],
        pattern=[[0, 2], [1, 128]],
        compare_op=ALU.is_ge, fill=B_SCH - 1.0e6, base=0,
        channel_multiplier=-1,
    )

    # ---- per-batch persistent sbuf ----
    x_td = [sb.tile([128, TC, 128], F32, tag=f"x_td{b}", name=f"x_td{b}")
            for b in range(B_PER)]
    x1bf = [sb.tile([128, TC, 128], BF16, tag=f"x1{b}", name=f"x1{b}")
            for b in range(B_PER)]
    x1T = [sb.tile([128, TC, 128], BF16, tag=f"x1T{b}", name=f"x1T{b}")
           for b in range(B_PER)]
    gT = [sb.tile([128, 4, T], BF16, tag=f"gT{b}", name=f"gT{b}")
          for b in range(B_PER)]
    oT = [sb.tile([128, TC, 128], BF16, tag=f"oT{b}", name=f"oT{b}")
          for b in range(B_PER)]

    xT_all = []

    def load_x(b):
        """x load + bf16 cast + transpose, all on DMA engines; emitted for
        both batches up front so batch 1's chain hides under batch 0."""
        xb = x_dram[b].rearrange("(c p) d -> p c d", p=128)
        nc.sync.dma_start(x_td[b], xb)
        xbf = sb.tile([128, TC, 128], BF16, tag=f"xbf{b}", name=f"xbf{b}")
        nc.gpsimd.dma_start(xbf, xb)
        xT = sb.tile([128, TC, 128], BF16, tag=f"xT{b}", name=f"xT{b}")
        nc.sync.dma_start(xT, xbf, transpose=True)
        xT_all.append(xT)

    def attn_core(b, weave=None):
        """QKV + per-uc (scores, exp, mask, av).  Calls weave(uc) after each
        u-chunk so the other batch's tail work can be interleaved."""
        xTf = xT_all[b].rearrange("p c t -> p (c t)")

        # qT / kT in 32-strip padded layout: head h=4g+j at partitions 32j
        qT = []
        kT = []
        for g in range(G):
            qp = ps.tile([128, T], F32, tag="s", bufs=2)
            for tb in range(2):
                nc.tensor.matmul(
                    qp[:, tb * 512:(tb + 1) * 512],
                    lhsT=wq_sb[:, g * 128:(g + 1) * 128],
                    rhs=xTf[:, tb * 512:(tb + 1) * 512],
                    start=True, stop=True,
                )
            qs = sb.tile([128, T], BF16, tag=f"qT{g}", name=f"qT{g}")
            nc.scalar.copy(out=qs, in_=qp)
            qT.append(qs)
            kp = ps.tile([128, T], F32, tag="s", bufs=2)
            for tb in range(2):
                nc.tensor.matmul(
                    kp[:, tb * 512:(tb + 1) * 512],
                    lhsT=wk_sb[:, g * 128:(g + 1) * 128],
                    rhs=xTf[:, tb * 512:(tb + 1) * 512],
                    start=True, stop=True,
                )
            ks = sb.tile([128, T], BF16, tag=f"kT{g}", name=f"kT{g}")
            # k eviction on ACT (Copy needs no table) to offload DVE
            nc.scalar.copy(out=ks, in_=kp)
            kT.append(ks)

        # v in [t, h*16+s] layout -> v' [u-chunk][h][17] bf16 with ones col
        vq = sb.tile([128, TC, H, 17], BF16, tag="vq")
        nc.vector.memset(vq[:, :, :, 16:17], 1.0)
        vp = ps.tile([128, T], F32, tag="s", bufs=2)
        for c in range(TC):
            # one accumulation group per PSUM bank (4 chunks of 128 cols);
            # start=True zeroes the whole bank, later chunks add onto zeros
            nc.tensor.matmul(
                vp[:, c * 128:(c + 1) * 128],
                lhsT=xTf[:, c * 128:(c + 1) * 128],
                rhs=wv_sb,
                start=(c % 4 == 0), stop=(c % 4 == 3),
                skip_group_check=True,
            )
        vsrc = vp.rearrange("p (c h s) -> p c h s", c=TC, h=H)
        nc.scalar.copy(out=vq[:, :, :, 0:16], in_=vsrc)

        # o (normalized attention output) accumulates here, then XBAR
        # transposes to oT in two 4-chunk pieces
        o_all = sb.tile([128, TC, 128], BF16, tag="o_all")

        E = [[None] * TC for _ in range(G)]

        def s_exp(g, uc):
            t0 = uc * 128
            tcols = T - t0
            e = eb.tile([128, 4, tcols], BF16, tag=f"E{g}_{uc}",
                        name=f"E{g}_{uc}")
            E[g][uc] = e
            for (pofs, plen) in _pieces(tcols):
                # head-pair score tiles: 2 banks each so the "s" tag can
                # double-buffer (scores of the next chunk overlap this exp)
                for p in range(2):
                    dve = _use_dve_exp(b, g, uc, p)
                    sp = ps.tile([128, 2, 512], F32, tag="s", bufs=2)
                    for jj in range(2):
                        j = 2 * p + jj
                        nc.tensor.matmul(
                            sp[:, jj, 0:plen],
                            lhsT=kT[g][32 * j:32 * j + 16, t0:t0 + 128],
                            rhs=qT[g][32 * j:32 * j + 16,
                                      t0 + pofs:t0 + pofs + plen],
                            start=True, stop=True,
                            tile_position=(32 * j, 0),
                        )
                    dst = e[:, 2 * p:2 * p + 2, pofs:pofs + plen]
                    if dve:
                        # bit-trick exp; the bias tensor also applies the
                        # causal diag mask via int16 saturation -> bf16 -0.0
                        bias = (maskb[:, :, 0:plen] if pofs == 0
                                else maskb[:, :, 128:128 + plen])
                        nc.vector.scalar_tensor_tensor(
                            out=dst.bitcast(I16), in0=sp[:, :, 0:plen],
                            scalar=A_SCH, in1=bias,
                            op0=ALU.mult, op1=ALU.add,
                        )
                    else:
                        nc.scalar.activation(
                            out=dst, in_=sp[:, :, 0:plen],
                            func=AF.Exp, scale=0.25,
                        )
                        if pofs == 0:
                            # causal mask on this head-pair's diagonal
                            # 128-block: keep where t_local >= u_local
                            # (partition index); only ACT halves need this
                            # (DVE halves mask via the bias tensor)
                            nc.gpsimd.affine_select(
                                out=e[:, 2 * p:2 * p + 2, 0:128],
                                in_=e[:, 2 * p:2 * p + 2, 0:128],
                                pattern=[[0, 2], [1, 128]],
                                compare_op=ALU.is_ge, fill=0.0, base=0,
                                channel_multiplier=-1,
                            )

        ops = {}

        def av_old(tcb):
            """Attention@v contributions from u-chunks < tcb: these depend
            only on already-finished E chunks, so PE runs them while ACT/DVE
            compute exp(tcb)."""
            op = ps.tile([128, H, 17], F32, tag="o", bufs=2)
            ops[tcb] = op
            for uc in range(tcb):
                ofs = (tcb - uc) * 128
                for g in range(G):
                    for j in range(4):
                        h = 4 * g + j
                        nc.tensor.matmul(
                            op[:, h, :],
                            lhsT=E[g][uc][:, j, ofs:ofs + 128],
                            rhs=vq[:, uc, h, :],
                            start=(uc == 0 and h == 0),
                            stop=False,
                            skip_group_check=True,
                        )

        def av_fin(tcb):
            """Diagonal-chunk contributions (need exp(tcb)+mask) and the
            softmax normalization.  Emitted AFTER the next chunk's scores so
            the in-order PE stream never stalls on exp latency."""
            op = ops.pop(tcb)
            for g in range(G):
                for j in range(4):
                    h = 4 * g + j
                    nc.tensor.matmul(
                        op[:, h, :],
                        lhsT=E[g][tcb][:, j, 0:128],
                        rhs=vq[:, tcb, h, :],
                        start=(tcb == 0 and h == 0),
                        stop=(h == H - 1),
                        skip_group_check=True,
                    )
            recip8 = sb.tile([128, H], F32, tag="recip8")
            nc.vector.reciprocal(recip8, op[:, :, 16])
            o_blk = o_all[:, tcb, :].rearrange("p (h s) -> p h s", h=H)
            nc.vector.tensor_mul(
                o_blk, op[:, :, 0:16], recip8.broadcast_to([128, H, 16])
            )
            if tcb == 3 or tcb == 7:
                c0 = tcb - 3
                nc.sync.dma_start(oT[b][:, c0:c0 + 4, :],
                                  o_all[:, c0:c0 + 4, :], transpose=True)

        for uc in range(TC):
            for g in range(G):
                s_exp(g, uc)
            av_old(uc)
            if uc > 0:
                av_fin(uc - 1)
            if weave is not None:
                weave(uc - 1)
        av_fin(TC - 1)
        if weave is not None:
            weave(TC - 1)

    def rsqrt_rows(vsrc, tagp):
        """rstd = 1/sqrt(vsrc + EPS) on Pool via the f32 bit trick plus two
        Newton steps (final rel err ~4e-6).  Keeps Ln/Exp off ACT: with this
        compiler's activation tables Ln and Exp live in different table sets,
        so each ACT-based rstd cost two 1283ns table loads."""
        ve = sb.tile([128, TC], F32, tag=tagp + "ve", name=tagp + "ve")
        nc.gpsimd.tensor_scalar(out=ve, in0=vsrc, scalar1=1.0, scalar2=EPS,
                                op0=ALU.mult, op1=ALU.add)
        y = sb.tile([128, TC], F32, tag=tagp + "y", name=tagp + "y")
        nc.gpsimd.tensor_scalar(
            out=y.bitcast(I32), in0=ve.bitcast(I32),
            scalar1=-0.5, scalar2=float(0x5F3759DF),
            op0=ALU.mult, op1=ALU.add,
        )
        t = sb.tile([128, TC], F32, tag=tagp + "t", name=tagp + "t")
        for _ in range(2):
            # y <- y * (1.5 - 0.5 * ve * y^2), Pool-supported ops only
            # (scalar_tensor_tensor is DVE-only on this core version)
            nc.gpsimd.tensor_mul(t, y, y)
            nc.gpsimd.tensor_mul(t, t, ve)
            nc.gpsimd.tensor_scalar(
                out=t, in0=t, scalar1=-0.5, scalar2=1.5,
                op0=ALU.mult, op1=ALU.add)
            nc.gpsimd.tensor_mul(y, t, y)
        return y

    def tail_pieces(b):
        """Emission closures for proj+LN1+MLP+LN2, in dependency order."""
        res1 = sb.tile([128, TC, 128], F32, tag="res1", name=f"res1_{b}")
        bn6 = sb.tile([128, TC, 6], F32, tag="bn6", name=f"bn6_{b}")
        mv = sb.tile([128, TC, 2], F32, tag="mv", name=f"mv_{b}")
        res2 = sb.tile([128, TC, 128], F32, tag="res2", name=f"res2_{b}")
        bn6b = sb.tile([128, TC, 6], F32, tag="bn6b", name=f"bn6b_{b}")
        mvb = sb.tile([128, TC, 2], F32, tag="mvb", name=f"mvb_{b}")

        def proj(c0):
            def f():
                for c in range(c0, c0 + 4):
                    pp = ps.tile([128, 128], F32, tag="hp", bufs=2)
                    nc.tensor.matmul(
                        pp, lhsT=oT[b][:, c, :], rhs=wp_sb,
                        start=True, stop=True,
                    )
                    nc.vector.tensor_add(res1[:, c, :], pp, x_td[b][:, c, :])
                    nc.vector.bn_stats(out=bn6[:, c, :], in_=res1[:, c, :])
                    nc.vector.bn_aggr(out=mv[:, c, :], in_=bn6[:, c, :])
            return f

        def ln1_apply():
            rstd8 = rsqrt_rows(mv[:, :, 1], "r1")
            for c in range(TC):
                nc.gpsimd.tensor_scalar(
                    out=x1bf[b][:, c, :], in0=res1[:, c, :],
                    scalar1=mv[:, c, 0:1], scalar2=rstd8[:, c:c + 1],
                    op0=ALU.subtract, op1=ALU.mult,
                )

        def x1t(c0):
            def f():
                nc.sync.dma_start(x1T[b][:, c0:c0 + 4, :],
                                  x1bf[b][:, c0:c0 + 4, :], transpose=True)
            return f

        x1Tf = x1T[b].rearrange("p c t -> p (c t)")

        def h1(fc):
            def f():
                for tb in range(2):
                    hp = ps.tile([128, 512], F32, tag="hp", bufs=2)
                    nc.tensor.matmul(
                        hp,
                        lhsT=w1_sb[:, fc * 128:(fc + 1) * 128],
                        rhs=x1Tf[:, tb * 512:(tb + 1) * 512],
                        start=True, stop=True,
                    )
                    nc.scalar.activation(
                        out=gT[b][:, fc, tb * 512:(tb + 1) * 512], in_=hp,
                        func=AF.Gelu)
            return f

        def x2(c0):
            def f():
                for c in range(c0, c0 + 2):
                    xp = ps.tile([128, 128], F32, tag="hp", bufs=2)
                    for fc in range(4):
                        nc.tensor.matmul(
                            xp,
                            lhsT=gT[b][:, fc, c * 128:(c + 1) * 128],
                            rhs=w2_sb[:, fc, :],
                            start=(fc == 0), stop=(fc == 3),
                        )
                    nc.vector.tensor_add(res2[:, c, :], xp, x1bf[b][:, c, :])
                    nc.vector.bn_stats(out=bn6b[:, c, :], in_=res2[:, c, :])
                    nc.vector.bn_aggr(out=mvb[:, c, :], in_=bn6b[:, c, :])
            return f

        def ln2_out():
            rstd8 = rsqrt_rows(mvb[:, :, 1], "r2")
            out_sb = sb.tile([128, TC, 128], F32, tag="out_sb",
                             name=f"out_sb_{b}")
            for c in range(TC):
                nc.gpsimd.tensor_scalar(
                    out=out_sb[:, c, :], in0=res2[:, c, :],
                    scalar1=mvb[:, c, 0:1], scalar2=rstd8[:, c:c + 1],
                    op0=ALU.subtract, op1=ALU.mult,
                )
            nc.sync.dma_start(
                out_dram[b].rearrange("(c p) d -> p c d", p=128), out_sb)

        return [
            proj(0), proj(4), ln1_apply, x1t(0), x1t(4),
            h1(0), h1(1), h1(2), h1(3),
            x2(0), x2(2), x2(4), x2(6), ln2_out,
        ]

    # pipelined schedule: b0 attention; then b1 attention with b0's tail
    # pieces woven into its per-uc slots (plus b1's first proj block, whose
    # oT half is ready after b1's av(3)); then b0 leftovers + b1 tail.
    load_x(0)
    load_x(1)
    attn_core(0)
    t0 = tail_pieces(0)
    t1 = tail_pieces(1)
    sched = {0: [0], 1: [1], 2: [2], 3: [3], 4: [4, 5], 5: [6, 7],
             6: [8, 9], 7: [10, 11]}
    done = set()

    def weave(uc):
        for i in sched.get(uc, []):
            t0[i]()
            done.add(i)
        if uc == 5:
            t1[0]()  # b1 proj(0): oT[b1][0:4] ready after b1's av(3)

    attn_core(1, weave=weave)
    for i in range(len(t0)):
        if i not in done:
            t0[i]()
    for i, piece in enumerate(t1):
        if i != 0:
            piece()


# ---------------- host side ----------------

_CACHED = None


def _get_compiled():
    global _CACHED
    if _CACHED is None:
        _CACHED = build_block_kernel()
    return _CACHED


def _prep_weights(inputs):
    f32 = np.float32
    Wq = np.asarray(inputs["Wq"], f32)  # [H, D, HS]
    Wk = np.asarray(inputs["Wk"], f32)
    Wv = np.asarray(inputs["Wv"], f32)
    Wp = np.asarray(inputs["Wp"], f32)  # [H*HS, D]
    W1 = np.asarray(inputs["W1"], f32)  # [D, 4D]
    W2 = np.asarray(inputs["W2"], f32)  # [4D, D]

    bf16 = ml_dtypes.bfloat16

    def strip_pack(W):
        out = np.zeros((D, G * 128), f32)
        for h in range(H):
            g, j = divmod(h, 4)
            out[:, g * 128 + 32 * j: g * 128 + 32 * j + HS] = W[h]
        return out.astype(bf16)

    wq = strip_pack(Wq)
    wk = strip_pack(Wk)
    wv = Wv.transpose(1, 0, 2).reshape(D, H * HS).astype(bf16)
    w2 = W2.reshape(4, 128, D).transpose(1, 0, 2).astype(bf16)
    return {
        "wq": wq, "wk": wk, "wv": np.ascontiguousarray(wv),
        "wp": np.ascontiguousarray(Wp.astype(bf16)),
        "w1": np.ascontiguousarray(W1.astype(bf16)),
        "w2": np.ascontiguousarray(w2),
    }


def run(inputs, trace=False):
    x = np.asarray(inputs["x"], np.float32)
    assert x.shape == (B_FULL, T, D), x.shape
    w = _prep_weights(inputs)
    nc = _get_compiled()
    in_maps = []
    for c in range(N_CORES):
        m = {"x": np.ascontiguousarray(x[c * B_PER:(c + 1) * B_PER])}
        m.update(w)
        in_maps.append(m)
    res = run_bass_kernel_spmd(
        nc, in_maps, core_ids=list(range(N_CORES)), trace=trace
    )
    out = np.concatenate([res.results[c]["out"] for c in range(N_CORES)], axis=0)
    return out.astype(np.float32), res


def kernel(**inputs):
    out, _ = run(inputs)
    return out


def _make_timed_runner(nc, in_maps):
    """Cached single-exec jitted runner with device-resident inputs.
    Returns a zero-arg callable that executes the NEFF once and blocks."""
    import jax
    from jax.experimental.shard_map import shard_map
    from jax.sharding import Mesh, NamedSharding, PartitionSpec

    from concourse import bass2jax, mybir as mb

    bass2jax.install_neuronx_cc_hook()
    partition_name = (
        nc.partition_id_tensor.name if nc.partition_id_tensor else None
    )
    in_names, out_names, out_avals, zero_outs = [], [], [], []
    for alloc in nc.m.functions[0].allocations:
        if not isinstance(alloc, mb.MemoryLocationSet):
            continue
        name = alloc.memorylocations[0].name
        if alloc.kind == "ExternalInput":
            if name != partition_name:
                in_names.append(name)
        elif alloc.kind == "ExternalOutput":
            shape = tuple(alloc.tensor_shape)
            dtype = mb.dt.np(alloc.dtype)
            out_names.append(name)
            out_avals.append(jax.core.ShapedArray(shape, dtype))
            zero_outs.append(np.zeros(shape, dtype))
    n_params = len(in_names)
    bind_names = tuple(in_names + out_names + (
        [partition_name] if partition_name else []))

    def _body(*args):
        operands = list(args)
        if partition_name is not None:
            operands.append(bass2jax.partition_id_tensor())
        return tuple(bass2jax._bass_exec_p.bind(
            *operands,
            out_avals=tuple(out_avals),
            in_names=bind_names,
            out_names=tuple(out_names),
            lowering_input_output_aliases=(),
            sim_require_finite=False,
            sim_require_nnan=False,
            nc=nc,
        ))

    n_cores = len(in_maps)
    devices = jax.devices()[:n_cores]
    mesh = Mesh(np.asarray(devices), ("core",))
    nin = n_params + len(out_names)
    fn = jax.jit(shard_map(
        _body, mesh=mesh,
        in_specs=(PartitionSpec("core"),) * nin,
        out_specs=(PartitionSpec("core"),) * len(out_names),
        check_rep=False,
    ))
    sharding = NamedSharding(mesh, PartitionSpec("core"))
    dev_args = [
        jax.device_put(
            np.concatenate([np.asarray(in_maps[c][nm]) for c in
                            range(n_cores)], axis=0), sharding)
        for nm in in_names
    ] + [
        jax.device_put(
            np.zeros((n_cores * z.shape[0], *z.shape[1:]), z.dtype), sharding)
        for z in zero_outs
    ]

    def call():
        out = fn(*dev_args)
        jax.block_until_ready(out)
        return out

    return call


def bench_ns(inputs, reps=20, loop_a=1, loop_b=129):
    """Per-exec NEFF time measured on device: the kernel body runs inside a
    Tile For_i loop; difference two loop counts to cancel the RPC floor.
    loop_b=129 so the 128-iteration delta (~28ms) dominates the ±5ms
    run-to-run jitter of the axon RPC floor."""
    import time as _time

    x = np.asarray(inputs["x"], np.float32)
    w = _prep_weights(inputs)
    in_maps = []
    for c in range(N_CORES):
        m = {"x": np.ascontiguousarray(x[c * B_PER:(c + 1) * B_PER])}
        m.update(w)
        in_maps.append(m)

    def timeit(call):
        call()
        call()
        best = float("inf")
        vals = []
        for _ in range(reps):
            t0 = _time.perf_counter()
            call()
            dt = _time.perf_counter() - t0
            vals.append(dt)
            best = min(best, dt)
        return best, sorted(vals)

    walls = {}
    for loop_n in (loop_a, loop_b):
        nc = build_block_kernel(loop_n=loop_n)
        call = _make_timed_runner(nc, in_maps)
        walls[loop_n], _ = timeit(call)
    ns = (walls[loop_b] - walls[loop_a]) / (loop_b - loop_a) * 1e9
    return ns, walls


# revision 35
# speedup vs baseline: 6.8542x; 1.0530x over previous
"""Trainium2 Bass kernel for a post-LN transformer block.

Reference computation (per batch element):
  q,k,v = per-head projections of x            [T,D] x [H,D,HS]
  attn  = softmax(causal(q k^T / sqrt(HS)))
  o     = attn @ v, concat heads, @ Wp
  x     = LN(o + x)
  h     = gelu(x @ W1) @ W2
  out   = LN(h + x)

Sharding: pure data-parallel over batch. B=16 across 8 cores -> 2 batch
elements per core, weights replicated, no collectives.

Engine-balance strategy (per core):
  - softmax exp is the dominant cost (~T^2/2 * H elements). It is split
    between ACT (native Exp) and DVE (Schraudolph bit-trick exp: bf16 bits
    = int16(round(s * 2^7*log2(e)/4 + 127*128 - C)); f32->int16 convert
    saturates, so extreme scores land at -0.0).  Final-output rel err
    contribution measured ~5e-4 (residual path dilutes attention error).
  - all transposes (xT, oT, x1T) run as XBAR block-transposes on the DMA
    engines (SBUF->SBUF, bf16), not on PE+DVE.
  - x -> bf16 conversion is a casting SWDGE DMA on gpsimd.
  - LN applies and the final output scaling run on Pool (gpsimd).
  - causal diag-block mask: gpsimd affine_select (fill=0) on E after exp.
  - scores S^T per (head-group, u-chunk) with 4-way PE row tiling
    (tile_position=(32j,0), K=16); av uses the ones-column trick so
    softmax denominators fall out of the attention matmul.
"""

import sys
from contextlib import ExitStack

import numpy as np

for _p in ("/opt/trn_rl_repo", "/opt/pypackages"):
    if _p not in sys.path:
        sys.path.append(_p)

import ml_dtypes  # noqa: E402

import concourse.bacc as bacc  # noqa: E402
import concourse.tile as tile  # noqa: E402
from concourse import mybir  # noqa: E402
from concourse.bass_utils import run_bass_kernel_spmd  # noqa: E402

F32 = mybir.dt.float32
BF16 = mybir.dt.bfloat16
I16 = mybir.dt.int16
I32 = mybir.dt.int32
AF = mybir.ActivationFunctionType
ALU = mybir.AluOpType

B_FULL = 16
N_CORES = 8
B_PER = B_FULL // N_CORES  # 2
T = 1024
D = 128
H = 8
HS = 16
TC = T // 128  # 8 chunks of 128 tokens
G = 2  # head groups of 4 (32-partition strips)
EPS = 1e-5

# Schraudolph exp constants for bf16 bit patterns, including the 1/sqrt(HS)
# score scale: e = bits16(round(s * (2^7/ln2) * 0.25 + (127*128 - C)))
A_SCH = (128.0 / float(np.log(2.0))) * 0.25
B_SCH = 127.0 * 128.0 - 6.0


def _pieces(tcols):
    """Split tcols into chunks of <=512."""
    out = []
    start = 0
    while start < tcols:
        ln = min(512, tcols - start)
        out.append((start, ln))
        start += ln
    return out


def _use_dve_exp(b, g, uc, p):
    """Which exp instructions run on DVE (bit-trick) vs ACT (native Exp).
    The p-halves of every score tile split between the engines so ACT and
    DVE run concurrently within each u-chunk (chunk-level assignment makes
    the engines take turns instead).  Batch 1's uc>=4 run fully on DVE so
    the woven gelu block never interleaves with ACT exp (each interleave
    costs a 1283ns ACT table load).  (g0, uc0)'s p1 stays on ACT to even
    out the engine loads."""
    if b == 1 and uc >= 4:
        return True
    if p == 0:
        return False
    return not (uc == 0 and g == 0)


def build_block_kernel(loop_n=1):
    nc = bacc.Bacc(
        "TRN2",
        target_bir_lowering=False,
        debug=False,
        enable_asserts=False,
    )

    x_dram = nc.dram_tensor("x", [B_PER, T, D], F32, kind="ExternalInput").ap()
    wq_dram = nc.dram_tensor("wq", [D, G * 128], BF16, kind="ExternalInput").ap()
    wk_dram = nc.dram_tensor("wk", [D, G * 128], BF16, kind="ExternalInput").ap()
    wv_dram = nc.dram_tensor("wv", [D, 128], BF16, kind="ExternalInput").ap()
    wp_dram = nc.dram_tensor("wp", [128, D], BF16, kind="ExternalInput").ap()
    w1_dram = nc.dram_tensor("w1", [D, 512], BF16, kind="ExternalInput").ap()
    w2_dram = nc.dram_tensor("w2", [128, 4, D], BF16, kind="ExternalInput").ap()
    out_dram = nc.dram_tensor("out", [B_PER, T, D], F32, kind="ExternalOutput").ap()

    with tile.TileContext(nc) as tc:
        if loop_n == 1:
            with ExitStack() as ctx:
                _body(ctx, tc, x_dram, wq_dram, wk_dram, wv_dram, wp_dram,
                      w1_dram, w2_dram, out_dram)
        else:
            with tc.For_i(0, loop_n, 1):
                with ExitStack() as ctx:
                    _body(ctx, tc, x_dram, wq_dram, wk_dram, wv_dram,
                          wp_dram, w1_dram, w2_dram, out_dram)

    nc.compile()
    return nc


def _body(ctx, tc, x_dram, wq_dram, wk_dram, wv_dram, wp_dram, w1_dram,
          w2_dram, out_dram):
    nc = tc.nc

    const = ctx.enter_context(tc.tile_pool(name="const", bufs=1))
    sb = ctx.enter_context(tc.tile_pool(name="sb", bufs=1))
    eb = ctx.enter_context(tc.tile_pool(name="eb", bufs=1))
    # PSUM budget (8 banks): "s" = 2-bank slots x2 (score tiles; also the
    # [128,1024] q/k/v tiles during attention prep), "hp" = 1 bank x2 (h1
    # halves + proj/x2 outputs), "o" = 1 bank x2 (av accumulators).
    # The MLP/proj tiles deliberately do NOT share "s" so the woven tail of
    # one batch never blocks the other batch's score pipeline.
    ps = ctx.enter_context(tc.tile_pool(name="ps", bufs=1, space="PSUM"))

    # ---- constants ----
    wq_sb = const.tile([D, G * 128], BF16, tag="wq")
    nc.sync.dma_start(wq_sb, wq_dram)
    wk_sb = const.tile([D, G * 128], BF16, tag="wk")
    nc.sync.dma_start(wk_sb, wk_dram)
    wv_sb = const.tile([D, 128], BF16, tag="wv")
    nc.sync.dma_start(wv_sb, wv_dram)
    wp_sb = const.tile([128, D], BF16, tag="wp")
    nc.sync.dma_start(wp_sb, wp_dram)
    w1_sb = const.tile([D, 512], BF16, tag="w1")
    nc.sync.dma_start(w1_sb, w1_dram)
    w2_sb = const.tile([128, 4, D], BF16, tag="w2")
    nc.sync.dma_start(w2_sb, w2_dram)

    # Schraudolph bias tensor for the DVE exp: cols 0:128 carry an extra
    # -1e6 on sub-diagonal positions (t_local < u_local) so the f32->int16
    # saturation maps masked scores to -32768 = bf16 -0.0; cols 128:640 are
    # the plain bias for off-diagonal score pieces.
    maskb = const.tile([128, 2, 640], F32, tag="maskb")
    nc.vector.memset(maskb, B_SCH)
    nc.gpsimd.affine_select(
        out=maskb[:, :, 0:128], in_=maskb[:, :, 0:128],
        pattern=[[0, 2], [1, 128]],
        compare_op=ALU.is_ge, fill=B_SCH - 1.0e6, base=0,
        channel_multiplier=-1,
    )

    # ---- per-batch persistent sbuf ----
    x_td = [sb.tile([128, TC, 128], F32, tag=f"x_td{b}", name=f"x_td{b}")
            for b in range(B_PER)]
    x1bf = [sb.tile([128, TC, 128], BF16, tag=f"x1{b}", name=f"x1{b}")
            for b in range(B_PER)]
    x1T = [sb.tile([128, TC, 128], BF16, tag=f"x1T{b}", name=f"x1T{b}")
           for b in range(B_PER)]
    gT = [sb.tile([128, 4, T], BF16, tag=f"gT{b}", name=f"gT{b}")
          for b in range(B_PER)]
    oT = [sb.tile([128, TC, 128], BF16, tag=f"oT{b}", name=f"oT{b}")
          for b in range(B_PER)]

    xT_all = []

    def load_x(b):
        """x load + bf16 cast + transpose, all on DMA engines; emitted for
        both batches up front so batch 1's chain hides under batch 0."""
        xb = x_dram[b].rearrange("(c p) d -> p c d", p=128)
        nc.sync.dma_start(x_td[b], xb)
        xbf = sb.tile([128, TC, 128], BF16, tag=f"xbf{b}", name=f"xbf{b}")
        nc.gpsimd.dma_start(xbf, xb)
        xT = sb.tile([128, TC, 128], BF16, tag=f"xT{b}", name=f"xT{b}")
        nc.sync.dma_start(xT, xbf, transpose=True)
        xT_all.append(xT)

    def attn_core(b, weave=None):
        """QKV + per-uc (scores, exp, mask, av).  Calls weave(uc) after each
        u-chunk so the other batch's tail work can be interleaved."""
        xTf = xT_all[b].rearrange("p c t -> p (c t)")

        # qT / kT in 32-strip padded layout: head h=4g+j at partitions 32j
        qT = []
        kT = []
        for g in range(G):
            qp = ps.tile([128, T], F32, tag="s", bufs=2)
            for tb in range(2):
                nc.tensor.matmul(
                    qp[:, tb * 512:(tb + 1) * 512],
                    lhsT=wq_sb[:, g * 128:(g + 1) * 128],
                    rhs=xTf[:, tb * 512:(tb + 1) * 512],
                    start=True, stop=True,
                )
            qs = sb.tile([128, T], BF16, tag=f"qT{g}", name=f"qT{g}")
            nc.scalar.copy(out=qs, in_=qp)
            qT.append(qs)
            kp = ps.tile([128, T], F32, tag="s", bufs=2)
            for tb in range(2):
                nc.tensor.matmul(
                    kp[:, tb * 512:(tb + 1) * 512],
                    lhsT=wk_sb[:, g * 128:(g + 1) * 128],
                    rhs=xTf[:, tb * 512:(tb + 1) * 512],
                    start=True, stop=True,
                )
            ks = sb.tile([128, T], BF16, tag=f"kT{g}", name=f"kT{g}")
            # k eviction on ACT (Copy needs no table) to offload DVE
            nc.scalar.copy(out=ks, in_=kp)
            kT.append(ks)

        # v in [t, h*16+s] layout -> v' [u-chunk][h][17] bf16 with ones col
        vq = sb.tile([128, TC, H, 17], BF16, tag="vq")
        nc.vector.memset(vq[:, :, :, 16:17], 1.0)
        vp = ps.tile([128, T], F32, tag="s", bufs=2)
        for c in range(TC):
            # one accumulation group per PSUM bank (4 chunks of 128 cols);
            # start=True zeroes the whole bank, later chunks add onto zeros
            nc.tensor.matmul(
                vp[:, c * 128:(c + 1) * 128],
                lhsT=xTf[:, c * 128:(c + 1) * 128],
                rhs=wv_sb,
                start=(c % 4 == 0), stop=(c % 4 == 3),
                skip_group_check=True,
            )
        vsrc = vp.rearrange("p (c h s) -> p c h s", c=TC, h=H)
        nc.vector.tensor_copy(out=vq[:, :, :, 0:16], in_=vsrc)

        # o (normalized attention output) accumulates here, then XBAR
        # transposes to oT in two 4-chunk pieces
        o_all = sb.tile([128, TC, 128], BF16, tag="o_all")

        E = [[None] * TC for _ in range(G)]

        def s_exp(g, uc):
            t0 = uc * 128
            tcols = T - t0
            e = eb.tile([128, 4, tcols], BF16, tag=f"E{g}_{uc}",
                        name=f"E{g}_{uc}")
            E[g][uc] = e
            for (pofs, plen) in _pieces(tcols):
                # head-pair score tiles: 2 banks each so the "s" tag can
                # double-buffer (scores of the next chunk overlap this exp)
                for p in range(2):
                    dve = _use_dve_exp(b, g, uc, p)
                    sp = ps.tile([128, 2, 512], F32, tag="s", bufs=2)
                    for jj in range(2):
                        j = 2 * p + jj
                        nc.tensor.matmul(
                            sp[:, jj, 0:plen],
                            lhsT=kT[g][32 * j:32 * j + 16, t0:t0 + 128],
                            rhs=qT[g][32 * j:32 * j + 16,
                                      t0 + pofs:t0 + pofs + plen],
                            start=True, stop=True,
                            tile_position=(32 * j, 0),
                        )
                    dst = e[:, 2 * p:2 * p + 2, pofs:pofs + plen]
                    if dve:
                        # bit-trick exp; the bias tensor also applies the
                        # causal diag mask via int16 saturation -> bf16 -0.0
                        bias = (maskb[:, :, 0:plen] if pofs == 0
                                else maskb[:, :, 128:128 + plen])
                        nc.vector.scalar_tensor_tensor(
                            out=dst.bitcast(I16), in0=sp[:, :, 0:plen],
                            scalar=A_SCH, in1=bias,
                            op0=ALU.mult, op1=ALU.add,
                        )
                    else:
                        nc.scalar.activation(
                            out=dst, in_=sp[:, :, 0:plen],
                            func=AF.Exp, scale=0.25,
                        )
                        if pofs == 0:
                            # causal mask on this head-pair's diagonal
                            # 128-block: keep where t_local >= u_local
                            # (partition index); only ACT halves need this
                            # (DVE halves mask via the bias tensor)
                            nc.gpsimd.affine_select(
                                out=e[:, 2 * p:2 * p + 2, 0:128],
                                in_=e[:, 2 * p:2 * p + 2, 0:128],
                                pattern=[[0, 2], [1, 128]],
                                compare_op=ALU.is_ge, fill=0.0, base=0,
                                channel_multiplier=-1,
                            )

        ops = {}

        def av_old(tcb):
            """Attention@v contributions from u-chunks < tcb: these depend
            only on already-finished E chunks, so PE runs them while ACT/DVE
            compute exp(tcb)."""
            op = ps.tile([128, H, 17], F32, tag="o", bufs=2)
            ops[tcb] = op
            for uc in range(tcb):
                ofs = (tcb - uc) * 128
                for g in range(G):
                    for j in range(4):
                        h = 4 * g + j
                        nc.tensor.matmul(
                            op[:, h, :],
                            lhsT=E[g][uc][:, j, ofs:ofs + 128],
                            rhs=vq[:, uc, h, :],
                            start=(uc == 0 and h == 0),
                            stop=False,
                            skip_group_check=True,
                        )

        def av_fin(tcb):
            """Diagonal-chunk contributions (need exp(tcb)+mask) and the
            softmax normalization.  Emitted AFTER the next chunk's scores so
            the in-order PE stream never stalls on exp latency."""
            op = ops.pop(tcb)
            for g in range(G):
                for j in range(4):
                    h = 4 * g + j
                    nc.tensor.matmul(
                        op[:, h, :],
                        lhsT=E[g][tcb][:, j, 0:128],
                        rhs=vq[:, tcb, h, :],
                        start=(tcb == 0 and h == 0),
                        stop=(h == H - 1),
                        skip_group_check=True,
                    )
            recip8 = sb.tile([128, H], F32, tag="recip8")
            nc.vector.reciprocal(recip8, op[:, :, 16])
            o_blk = o_all[:, tcb, :].rearrange("p (h s) -> p h s", h=H)
            nc.vector.tensor_mul(
                o_blk, op[:, :, 0:16], recip8.broadcast_to([128, H, 16])
            )
            if tcb == 3 or tcb == 7:
                c0 = tcb - 3
                nc.sync.dma_start(oT[b][:, c0:c0 + 4, :],
                                  o_all[:, c0:c0 + 4, :], transpose=True)

        for uc in range(TC):
            for g in range(G):
                s_exp(g, uc)
            av_old(uc)
            if uc > 0:
                av_fin(uc - 1)
            if weave is not None:
                weave(uc - 1)
        av_fin(TC - 1)
        if weave is not None:
            weave(TC - 1)

    def rsqrt_rows(vsrc, tagp):
        """rstd = 1/sqrt(vsrc + EPS) on Pool via the f32 bit trick plus two
        Newton steps (final rel err ~4e-6).  Keeps Ln/Exp off ACT: with this
        compiler's activation tables Ln and Exp live in different table sets,
        so each ACT-based rstd cost two 1283ns table loads."""
        ve = sb.tile([128, TC], F32, tag=tagp + "ve", name=tagp + "ve")
        nc.gpsimd.tensor_scalar(out=ve, in0=vsrc, scalar1=1.0, scalar2=EPS,
                                op0=ALU.mult, op1=ALU.add)
        y = sb.tile([128, TC], F32, tag=tagp + "y", name=tagp + "y")
        nc.gpsimd.tensor_scalar(
            out=y.bitcast(I32), in0=ve.bitcast(I32),
            scalar1=-0.5, scalar2=float(0x5F3759DF),
            op0=ALU.mult, op1=ALU.add,
        )
        t = sb.tile([128, TC], F32, tag=tagp + "t", name=tagp + "t")
        for _ in range(2):
            # y <- y * (1.5 - 0.5 * ve * y^2), Pool-supported ops only
            # (scalar_tensor_tensor is DVE-only on this core version)
            nc.gpsimd.tensor_mul(t, y, y)
            nc.gpsimd.tensor_mul(t, t, ve)
            nc.gpsimd.tensor_scalar(
                out=t, in0=t, scalar1=-0.5, scalar2=1.5,
                op0=ALU.mult, op1=ALU.add)
            nc.gpsimd.tensor_mul(y, t, y)
        return y

    def tail_pieces(b):
        """Emission closures for proj+LN1+MLP+LN2, in dependency order."""
        res1 = sb.tile([128, TC, 128], F32, tag="res1", name=f"res1_{b}")
        bn6 = sb.tile([128, TC, 6], F32, tag="bn6", name=f"bn6_{b}")
        mv = sb.tile([128, TC, 2], F32, tag="mv", name=f"mv_{b}")
        res2 = sb.tile([128, TC, 128], F32, tag="res2", name=f"res2_{b}")
        bn6b = sb.tile([128, TC, 6], F32, tag="bn6b", name=f"bn6b_{b}")
        mvb = sb.tile([128, TC, 2], F32, tag="mvb", name=f"mvb_{b}")

        def proj(c0):
            def f():
                for c in range(c0, c0 + 4):
                    pp = ps.tile([128, 128], F32, tag="hp", bufs=2)
                    nc.tensor.matmul(
                        pp, lhsT=oT[b][:, c, :], rhs=wp_sb,
                        start=True, stop=True,
                    )
                    nc.vector.tensor_add(res1[:, c, :], pp, x_td[b][:, c, :])
                    nc.vector.bn_stats(out=bn6[:, c, :], in_=res1[:, c, :])
                    nc.vector.bn_aggr(out=mv[:, c, :], in_=bn6[:, c, :])
            return f

        def ln1_apply():
            rstd8 = rsqrt_rows(mv[:, :, 1], "r1")
            for c in range(TC):
                nc.gpsimd.tensor_scalar(
                    out=x1bf[b][:, c, :], in0=res1[:, c, :],
                    scalar1=mv[:, c, 0:1], scalar2=rstd8[:, c:c + 1],
                    op0=ALU.subtract, op1=ALU.mult,
                )

        def x1t(c0):
            def f():
                nc.sync.dma_start(x1T[b][:, c0:c0 + 4, :],
                                  x1bf[b][:, c0:c0 + 4, :], transpose=True)
            return f

        x1Tf = x1T[b].rearrange("p c t -> p (c t)")

        def h1(fc):
            def f():
                for tb in range(2):
                    hp = ps.tile([128, 512], F32, tag="hp", bufs=2)
                    nc.tensor.matmul(
                        hp,
                        lhsT=w1_sb[:, fc * 128:(fc + 1) * 128],
                        rhs=x1Tf[:, tb * 512:(tb + 1) * 512],
                        start=True, stop=True,
                    )
                    nc.scalar.activation(
                        out=gT[b][:, fc, tb * 512:(tb + 1) * 512], in_=hp,
                        func=AF.Gelu)
            return f

        def x2(c0):
            def f():
                for c in range(c0, c0 + 2):
                    xp = ps.tile([128, 128], F32, tag="hp", bufs=2)
                    for fc in range(4):
                        nc.tensor.matmul(
                            xp,
                            lhsT=gT[b][:, fc, c * 128:(c + 1) * 128],
                            rhs=w2_sb[:, fc, :],
                            start=(fc == 0), stop=(fc == 3),
                        )
                    nc.vector.tensor_add(res2[:, c, :], xp, x1bf[b][:, c, :])
                    nc.vector.bn_stats(out=bn6b[:, c, :], in_=res2[:, c, :])
                    nc.vector.bn_aggr(out=mvb[:, c, :], in_=bn6b[:, c, :])
            return f

        def ln2_out():
            rstd8 = rsqrt_rows(mvb[:, :, 1], "r2")
            out_sb = sb.tile([128, TC, 128], F32, tag="out_sb",
                             name=f"out_sb_{b}")
            for c in range(TC):
                nc.gpsimd.tensor_scalar(
                    out=out_sb[:, c, :], in0=res2[:, c, :],
                    scalar1=mvb[:, c, 0:1], scalar2=rstd8[:, c:c + 1],
                    op0=ALU.subtract, op1=ALU.mult,
                )
            nc.sync.dma_start(
                out_dram[b].rearrange("(c p) d -> p c d", p=128), out_sb)

        return [
            proj(0), proj(4), ln1_apply, x1t(0), x1t(4),
            h1(0), h1(1), h1(2), h1(3),
            x2(0), x2(2), x2(4), x2(6), ln2_out,
        ]

    # pipelined schedule: b0 attention (with its own first proj block woven
    # in once oT[b0][0:4] lands); then b1 attention with b0's remaining tail
    # pieces woven into its per-uc slots (plus b1's first proj block); then
    # b0 leftovers + b1 tail.
    load_x(0)
    load_x(1)
    t0 = tail_pieces(0)
    t1 = tail_pieces(1)

    def weave0(uc):
        if uc == 5:
            t0[0]()  # b0 proj(0): oT[b0][0:4] ready after b0's av_fin(3)

    attn_core(0, weave=weave0)
    sched = {0: [1], 1: [2], 2: [3], 3: [4], 4: [5], 5: [6, 7],
             6: [8, 9], 7: [10, 11]}
    done = {0}

    def weave(uc):
        for i in sched.get(uc, []):
            t0[i]()
            done.add(i)
        if uc == 5:
            t1[0]()  # b1 proj(0): oT[b1][0:4] ready after b1's av(3)

    attn_core(1, weave=weave)
    for i in range(len(t0)):
        if i not in done:
            t0[i]()
    for i, piece in enumerate(t1):
        if i != 0:
            piece()


# ---------------- host side ----------------

_CACHED = None


def _get_compiled():
    global _CACHED
    if _CACHED is None:
        _CACHED = build_block_kernel()
    return _CACHED


def _prep_weights(inputs):
    f32 = np.float32
    Wq = np.asarray(inputs["Wq"], f32)  # [H, D, HS]
    Wk = np.asarray(inputs["Wk"], f32)
    Wv = np.asarray(inputs["Wv"], f32)
    Wp = np.asarray(inputs["Wp"], f32)  # [H*HS, D]
    W1 = np.asarray(inputs["W1"], f32)  # [D, 4D]
    W2 = np.asarray(inputs["W2"], f32)  # [4D, D]

    bf16 = ml_dtypes.bfloat16

    def strip_pack(W):
        out = np.zeros((D, G * 128), f32)
        for h in range(H):
            g, j = divmod(h, 4)
            out[:, g * 128 + 32 * j: g * 128 + 32 * j + HS] = W[h]
        return out.astype(bf16)

    wq = strip_pack(Wq)
    wk = strip_pack(Wk)
    wv = Wv.transpose(1, 0, 2).reshape(D, H * HS).astype(bf16)
    w2 = W2.reshape(4, 128, D).transpose(1, 0, 2).astype(bf16)
    return {
        "wq": wq, "wk": wk, "wv": np.ascontiguousarray(wv),
        "wp": np.ascontiguousarray(Wp.astype(bf16)),
        "w1": np.ascontiguousarray(W1.astype(bf16)),
        "w2": np.ascontiguousarray(w2),
    }


def run(inputs, trace=False):
    x = np.asarray(inputs["x"], np.float32)
    assert x.shape == (B_FULL, T, D), x.shape
    w = _prep_weights(inputs)
    nc = _get_compiled()
    in_maps = []
    for c in range(N_CORES):
        m = {"x": np.ascontiguousarray(x[c * B_PER:(c + 1) * B_PER])}
        m.update(w)
        in_maps.append(m)
    res = run_bass_kernel_spmd(
        nc, in_maps, core_ids=list(range(N_CORES)), trace=trace
    )
    out = np.concatenate([res.results[c]["out"] for c in range(N_CORES)], axis=0)
    return out.astype(np.float32), res


def kernel(**inputs):
    out, _ = run(inputs)
    return out


def _make_timed_runner(nc, in_maps):
    """Cached single-exec jitted runner with device-resident inputs.
    Returns a zero-arg callable that executes the NEFF once and blocks."""
    import jax
    from jax.experimental.shard_map import shard_map
    from jax.sharding import Mesh, NamedSharding, PartitionSpec

    from concourse import bass2jax, mybir as mb

    bass2jax.install_neuronx_cc_hook()
    partition_name = (
        nc.partition_id_tensor.name if nc.partition_id_tensor else None
    )
    in_names, out_names, out_avals, zero_outs = [], [], [], []
    for alloc in nc.m.functions[0].allocations:
        if not isinstance(alloc, mb.MemoryLocationSet):
            continue
        name = alloc.memorylocations[0].name
        if alloc.kind == "ExternalInput":
            if name != partition_name:
                in_names.append(name)
        elif alloc.kind == "ExternalOutput":
            shape = tuple(alloc.tensor_shape)
            dtype = mb.dt.np(alloc.dtype)
            out_names.append(name)
            out_avals.append(jax.core.ShapedArray(shape, dtype))
            zero_outs.append(np.zeros(shape, dtype))
    n_params = len(in_names)
    bind_names = tuple(in_names + out_names + (
        [partition_name] if partition_name else []))

    def _body(*args):
        operands = list(args)
        if partition_name is not None:
            operands.append(bass2jax.partition_id_tensor())
        return tuple(bass2jax._bass_exec_p.bind(
            *operands,
            out_avals=tuple(out_avals),
            in_names=bind_names,
            out_names=tuple(out_names),
            lowering_input_output_aliases=(),
            sim_require_finite=False,
            sim_require_nnan=False,
            nc=nc,
        ))

    n_cores = len(in_maps)
    devices = jax.devices()[:n_cores]
    mesh = Mesh(np.asarray(devices), ("core",))
    nin = n_params + len(out_names)
    fn = jax.jit(shard_map(
        _body, mesh=mesh,
        in_specs=(PartitionSpec("core"),) * nin,
        out_specs=(PartitionSpec("core"),) * len(out_names),
        check_rep=False,
    ))
    sharding = NamedSharding(mesh, PartitionSpec("core"))
    dev_args = [
        jax.device_put(
            np.concatenate([np.asarray(in_maps[c][nm]) for c in
                            range(n_cores)], axis=0), sharding)
        for nm in in_names
    ] + [
        jax.device_put(
            np.zeros((n_cores * z.shape[0], *z.shape[1:]), z.dtype), sharding)
        for z in zero_outs
    ]

    def call():
        out = fn(*dev_args)
        jax.block_until_ready(out)
        return out

    return call


def bench_ns(inputs, reps=20, loop_a=1, loop_b=129):
    """Per-exec NEFF time measured on device: the kernel body runs inside a
    Tile For_i loop; difference two loop counts to cancel the RPC floor.
    loop_b=129 so the 128-iteration delta (~28ms) dominates the ±5ms
    run-to-run jitter of the axon RPC floor."""
    import time as _time

    x = np.asarray(inputs["x"], np.float32)
    w = _prep_weights(inputs)
    in_maps = []
    for c in range(N_CORES):
        m = {"x": np.ascontiguousarray(x[c * B_PER:(c + 1) * B_PER])}
        m.update(w)
        in_maps.append(m)

    def timeit(call):
        call()
        call()
        best = float("inf")
        vals = []
        for _ in range(reps):
            t0 = _time.perf_counter()
            call()
            dt = _time.perf_counter() - t0
            vals.append(dt)
            best = min(best, dt)
        return best, sorted(vals)

    walls = {}
    for loop_n in (loop_a, loop_b):
        nc = build_block_kernel(loop_n=loop_n)
        call = _make_timed_runner(nc, in_maps)
        walls[loop_n], _ = timeit(call)
    ns = (walls[loop_b] - walls[loop_a]) / (loop_b - loop_a) * 1e9
    return ns, walls
